# revision 50
# baseline (speedup 1.0000x reference)
"""Trainium2 Bass kernel for CollisionDistanceEvaluator (segment_reduce).

Contract: kernel(**inputs) takes FULL inputs (trans [4096,3] f32,
quat [4096,4] f32, pc [4096,4096,3] f32) and returns the FULL output
[4096,1] f32, running the heavy per-point work on 8 NeuronCores
(pure data-parallel over the batch dim, 512 batches/core).

Math: reference rotates pc by inv(quat) (unit norm -> rotation R),
translates by -trans, tests an axis-aligned box (center C, half ext H),
and takes the per-batch masked mean of point norms.  Host precomputes
    A[b] = R[b] / H[:,None]       (box-normalized rotation rows)
    o[b] = -(trans[b] + C) / H
Device, per point p (v_i = A_i.p + o_i, the box coords):
    mask = max(v_x^2, v_y^2, |v_z|) <= 1
    n2/Hx^2 = v_x^2 + ky*v_y^2 + kz*(v_z^2 + (2Cz/Hz)*v_z) + cc
    out[b]  = -10000*Hx*sum(mask*sqrt(n2~))/max(cnt,1)  (+10000 if cnt==0)

Engine split per 128-batch tile, per 1024-pt chunk (4 tiles x 4 chunks):
  PE     : 18 diag matmuls (z,x,y plane order) -> PSUM
  ACT    : vz = Identity(uz+oz); qx,qy = Square(u+o); Sqrt(mn)+accum
           (sqrt issued one chunk late so ACT never blocks on DVE)
  DVE    : max(qx,qy); abs_max(.,vz); 3x stt n2-chain; mn stt
  GPSIMD : qz = vz*vz (tensor_tensor); mask = is_le(mx,1) + count accum
"""

import numpy as np

import concourse.bass as bass
import concourse.bacc as bacc
import concourse.mybir as mybir
from concourse.tile import TileContext
from concourse.bass_utils import run_bass_kernel_spmd
from concourse import library_config


def _ensure_ntff_hook():
    """Register the axon NTFF profile hook if the image's antenv lacks it."""
    import sys
    import types
    try:
        from antenv.axon_hooks import get_axon_ntff_profile_hook  # noqa
        return
    except ImportError:
        pass
    try:
        import antenv
        from trn_agent_boot.trn_boot import _ntff_profile_via_ctypes
        mod = types.ModuleType("antenv.axon_hooks")
        mod._hook = _ntff_profile_via_ctypes("/opt/axon/libaxon_pjrt.so")

        def set_axon_ntff_profile_hook(h):
            mod._hook = h

        def get_axon_ntff_profile_hook():
            return mod._hook

        mod.set_axon_ntff_profile_hook = set_axon_ntff_profile_hook
        mod.get_axon_ntff_profile_hook = get_axon_ntff_profile_hook
        sys.modules["antenv.axon_hooks"] = mod
        antenv.axon_hooks = mod
    except Exception:
        pass


_ensure_ntff_hook()

N_CORES = 8
B_FULL, N_PTS = 4096, 4096
B_CORE = B_FULL // N_CORES          # 512
N_TILES = B_CORE // 128             # 4
CHUNK = 1024                         # points per PSUM/drain chunk
N_CHUNKS = N_PTS // CHUNK           # 4
PAIR = 2 * CHUNK                     # DVE/sqrt operate pair-wide
N_PAIRS = N_PTS // PAIR             # 2
MM = 512                             # matmul moving max

DIST_THRESHOLD = 0.001
DIST_COEFF = 10000.0
BOX_CENTER = np.array([-0.001782, 1.005e-05, 0.0431621], dtype=np.float64)
HALF_EXT = np.array([
    0.204416 / 2 + DIST_THRESHOLD,
    0.0632517 / 2 + DIST_THRESHOLD,
    0.1381738 / 2 + DIST_THRESHOLD,
], dtype=np.float64)

# n2 normalization constants (n2 / Hx^2)
_HX2 = HALF_EXT[0] * HALF_EXT[0]
K_Y = float((HALF_EXT[1] / HALF_EXT[0]) ** 2)
K_Z = float((HALF_EXT[2] / HALF_EXT[0]) ** 2)
KL = float(2.0 * HALF_EXT[2] * BOX_CENTER[2] / _HX2)  # v_z linear coeff
HZHX = float(HALF_EXT[2] / HALF_EXT[0])
EPS_PAD = 1.5e-3
CC = float((BOX_CENTER[0] ** 2 + BOX_CENTER[1] ** 2 + BOX_CENTER[2] ** 2)
           / _HX2 + EPS_PAD)
CC_XY = float((BOX_CENTER[0] ** 2 + BOX_CENTER[1] ** 2) / _HX2 + EPS_PAD)
OUT_SCALE = float(-DIST_COEFF * HALF_EXT[0])

_NC_CACHE = {}


def _build_bass():
    f16, f32 = mybir.dt.float16, mybir.dt.float32
    Alu = mybir.AluOpType
    Act = mybir.ActivationFunctionType

    nc = bacc.Bacc()
    xyz = nc.declare_dram_parameter(
        "xyz", [3, B_CORE, N_PTS], f16, isOutput=False)
    # per-batch scalars: cols 0-2 = o_i
    coef = nc.declare_dram_parameter(
        "coef", [N_TILES, 128, 8], f32, isOutput=False)
    # diagonal PE weights: [tile, i, c] -> diag(A[batch, i, c])
    wd = nc.declare_dram_parameter(
        "wd", [N_TILES, 128, 9 * 128], f16, isOutput=False)
    # out[p, t] = output of batch (t*128 + p); host transposes
    out = nc.declare_dram_parameter("out", [128, N_TILES], f16, isOutput=True)
    xyz_ap, coef_ap, wd_ap, out_ap = xyz[:], coef[:], wd[:], out[:]

    with TileContext(nc) as tc, \
            tc.tile_pool(name="data", bufs=2) as data, \
            tc.tile_pool(name="ck", bufs=3) as ck, \
            tc.tile_pool(name="wpool", bufs=2) as wpool, \
            tc.tile_pool(name="psum", bufs=1, space="PSUM") as psum, \
            tc.tile_pool(name="small", bufs=4) as small:
        # PE warm-up: junk matmuls during the initial DMA ramp keep the
        # HAM clock gate open so the first real matmuls run at 2.4 GHz
        wu = wpool.tile([128, 128], f16, tag="warm")
        nc.vector.memset(wu[:], 0.0)
        res16 = small.tile([128, N_TILES], f16, tag="res")
        pwu = psum.tile([128, CHUNK], f32, tag="uza")
        for k in range(32):
            nc.tensor.matmul(pwu[:, :128], wu[:], wu[:, :],
                             start=True, stop=True)

        for t in range(N_TILES):
            r = slice(t * 128, (t + 1) * 128)
            ct = small.tile([128, 8], f32, tag="coef")
            nc.sync.dma_start(out=ct[:], in_=coef_ap[t, :, :])
            xt = data.tile([128, N_PTS], f16, tag="x")
            yt = data.tile([128, N_PTS], f16, tag="y")
            zt = data.tile([128, N_PTS], f16, tag="z")
            # weights first (gate the first LDWEIGHTS), then the first
            # chunk's columns so compute starts early
            wt = wpool.tile([128, 9 * 128], f16, tag="wt")
            (nc.scalar if t == 0 else nc.sync).dma_start(
                out=wt[:], in_=wd_ap[t, :, :])
            q0 = slice(0, CHUNK)
            (nc.scalar if t == 0 else nc.sync).dma_start(
                out=zt[:, q0], in_=xyz_ap[2, r, q0])
            nc.sync.dma_start(out=xt[:, q0], in_=xyz_ap[0, r, q0])
            nc.sync.dma_start(out=yt[:, q0], in_=xyz_ap[1, r, q0])
            q1 = slice(CHUNK, 2 * CHUNK)
            nc.sync.dma_start(out=zt[:, q1], in_=xyz_ap[2, r, q1])
            nc.sync.dma_start(out=xt[:, q1], in_=xyz_ap[0, r, q1])
            nc.sync.dma_start(out=yt[:, q1], in_=xyz_ap[1, r, q1])
            h1 = slice(2 * CHUNK, N_PTS)
            nc.sync.dma_start(out=zt[:, h1], in_=xyz_ap[2, r, h1])
            nc.sync.dma_start(out=xt[:, h1], in_=xyz_ap[0, r, h1])
            nc.sync.dma_start(out=yt[:, h1], in_=xyz_ap[1, r, h1])
            planes = (xt, yt, zt)
            if t == 0:
                # keep the PE clock hot while the first DMAs land (a
                # >3.4us PE idle gap re-throttles the HAM clock)
                for k in range(56):
                    nc.tensor.matmul(pwu[:, :128], wu[:], wu[:, :],
                                     start=True, stop=True)

            def wsel(i, c, wt=wt):
                w = 3 * i + c
                return wt[:, w * 128:(w + 1) * 128]

            rcv = small.tile([128, N_CHUNKS], f32, tag="rcv")
            rsv = small.tile([128, N_CHUNKS], f32, tag="rsv")

            pend = []  # sqrt runs 1 stage late so ACT never stalls on DVE
            slot = [0]

            def flush_sqrt(keep):
                while len(pend) > keep:
                    pmn, ss = pend.pop(0)
                    w = pmn.shape[-1]
                    sq = ck.tile([128, PAIR], f16, tag="mx")
                    nc.scalar.activation(
                        sq[:, :w], pmn, Act.Sqrt,
                        accum_out=rsv[:, ss:ss + 1])

            def dve_stage(w, odd, qx, qy, qz, vz, sz, mn):
                # DVE mask chain + count accum on slice w of the pair
                if not odd:
                    nc.vector.tensor_tensor(qz[:, w], vz[:, w], vz[:, w],
                                            Alu.mult)
                mx = ck.tile([128, PAIR], f16, tag="mx")
                wd_ = w.stop - w.start
                mxv = mx[:, :wd_]
                nc.vector.tensor_tensor(mxv, qx[:, w], qy[:, w], Alu.max)
                nc.vector.tensor_tensor(mxv, mxv, qz[:, w], Alu.max)
                g = ck.tile([128, PAIR], f16, tag="g")
                gv = g[:, :wd_]
                ss = slot[0]
                slot[0] += 1
                nc.vector.tensor_scalar(
                    gv, mxv, 1.0, 0.0, Alu.is_le, Alu.add,
                    accum_out=rcv[:, ss:ss + 1])
                # DVE n2 assembly (4x ts + 2x TT):
                # even: n2~ = qx + ky*qy + kz*qz + kl*vz + cc
                # odd : n2~ = qx + (ky*qy + ccxy) + sz
                t1 = ck.tile([128, PAIR], f16, tag="t1")
                t1v = t1[:, :wd_]
                if odd:
                    nc.vector.tensor_scalar(
                        t1v, qy[:, w], K_Y, CC_XY, Alu.mult, Alu.add)
                    nc.vector.tensor_tensor(t1v, qx[:, w], t1v, Alu.add)
                    nc.vector.tensor_tensor(t1v, t1v, sz[:, w], Alu.add)
                else:
                    nc.vector.tensor_scalar(
                        t1v, qy[:, w], K_Y, None, Alu.mult)
                    t2 = ck.tile([128, PAIR], f16, tag="t2")
                    t2v = t2[:, :wd_]
                    s3 = ck.tile([128, PAIR], f16, tag="s3")
                    s3v = s3[:, :wd_]
                    nc.vector.tensor_scalar(
                        t2v, qz[:, w], K_Z, None, Alu.mult)
                    nc.vector.tensor_scalar(
                        s3v, vz[:, w], KL, CC, Alu.mult, Alu.add)
                    nc.vector.tensor_tensor(t1v, qx[:, w], t1v, Alu.add)
                    nc.vector.tensor_tensor(t2v, t2v, s3v, Alu.add)
                    nc.vector.tensor_tensor(t1v, t1v, t2v, Alu.add)
                # masked n2 (g is the 0/1 mask)
                mn = ck.tile([128, PAIR], f16, tag="mn")
                mnv = mn[:, :wd_]
                nc.vector.tensor_tensor(mnv, gv, t1v, Alu.mult)
                pend.append((mnv, ss))

            for p in range(N_PAIRS):
                odd = (t * N_PAIRS + p) % 2 == 1
                # first pair of the first tile and last pair of the last
                # tile run the DVE stage per chunk: shorter ramp/tail
                split = (t == 0 and p == 0) or \
                        (t == N_TILES - 1 and p == N_PAIRS - 1)
                # pair-wide drained tiles, filled chunk-halves at a time
                qx = ck.tile([128, PAIR], f16, tag="qx")
                qy = ck.tile([128, PAIR], f16, tag="qy")
                qz = ck.tile([128, PAIR], f16, tag="qz")
                sz = vz = None
                if odd:
                    sz = ck.tile([128, PAIR], f16, tag="t2")
                else:
                    vz = ck.tile([128, PAIR], f16, tag="vz")
                for h2 in range(2):
                    j = 2 * p + h2
                    d = slice(h2 * CHUNK, (h2 + 1) * CHUNK)
                    uz = psum.tile([128, CHUNK], f32,
                                   tag="uza" if j % 2 == 0 else "uzb")
                    ux = psum.tile([128, CHUNK], f32, tag="ux")
                    uy = psum.tile([128, CHUNK], f32, tag="uy")
                    # PE: z first (its drains feed the DVE chain); c outer
                    # so each diag weight loads once per chunk (9 LDW)
                    for ut, i in ((uz, 2), (ux, 0), (uy, 1)):
                        for c in range(3):
                            for h in range(CHUNK // MM):
                                hs = slice(h * MM, (h + 1) * MM)
                                ps = slice(j * CHUNK + h * MM,
                                           j * CHUNK + (h + 1) * MM)
                                nc.tensor.matmul(
                                    ut[:, hs], wsel(i, c),
                                    planes[c][:, ps],
                                    start=(c == 0), stop=(c == 2))
                    # ACT drains (PSUM -> SBUF f16).  Even pairs: z as
                    # Identity (DVE squares it); odd pairs: z drained twice
                    # on ACT (qz mask + sz = exact z term of n2~) so DVE
                    # and ACT loads balance.
                    if odd:
                        nc.scalar.activation(
                            qz[:, d], uz[:], Act.Square, bias=ct[:, 2:3])
                        nc.scalar.activation(
                            sz[:, d], uz[:], Act.Square, bias=ct[:, 3:4],
                            scale=HZHX)
                    else:
                        nc.scalar.activation(
                            vz[:, d], uz[:], Act.Identity, bias=ct[:, 2:3])
                    nc.scalar.activation(
                        qx[:, d], ux[:], Act.Square, bias=ct[:, 0:1])
                    nc.scalar.activation(
                        qy[:, d], uy[:], Act.Square, bias=ct[:, 1:2])
                    if split:
                        dve_stage(d, odd, qx, qy, qz, vz, sz, None)
                    if h2 == 0:
                        flush_sqrt(1)
                if not split:
                    dve_stage(slice(0, PAIR), odd, qx, qy, qz, vz,
                              sz, None)
            flush_sqrt(0)

            # tail: out = (cnt==0)*10000 + OUT_SCALE*rs/max(cnt,1)
            rc = small.tile([128, 1], f32, tag="rc")
            rs = small.tile([128, 1], f32, tag="rs")
            ns = slot[0]
            nc.vector.tensor_reduce(
                rc[:], rcv[:, 0:ns], mybir.AxisListType.X, Alu.add)
            nc.vector.tensor_reduce(
                rs[:], rsv[:, 0:ns], mybir.AxisListType.X, Alu.add)
            rc1 = small.tile([128, 1], f32, tag="rc1")
            nc.vector.tensor_scalar(rc1[:], rc[:], 1.0, None, Alu.max)
            inv = small.tile([128, 1], f32, tag="inv")
            nc.vector.reciprocal(inv[:], rc1[:])
            val = small.tile([128, 1], f32, tag="val")
            nc.vector.scalar_tensor_tensor(
                val[:], rs[:], OUT_SCALE, inv[:], Alu.mult, Alu.mult)
            zer = small.tile([128, 1], f32, tag="zer")
            nc.vector.tensor_scalar(zer[:], rc[:], 0.0, None, Alu.is_le)
            nc.vector.scalar_tensor_tensor(
                res16[:, t:t + 1], zer[:], DIST_COEFF, val[:],
                Alu.mult, Alu.add)

        # one packed result store (vs four 128-descriptor scatter DMAs)
        nc.sync.dma_start(out=out_ap[:, :], in_=res16[:])
    nc.compile()
    return nc


def _get_nc():
    if "nc" not in _NC_CACHE:
        _NC_CACHE["nc"] = _build_bass()
    return _NC_CACHE["nc"]


def _host_coefficients(trans, quat):
    """Per-batch A = R/H [B,3,3] and o = -(t+C)/H [B,3] (computed in f64)."""
    q = np.asarray(quat, np.float64)
    t = np.asarray(trans, np.float64)
    B = q.shape[0]
    s = (q * q).sum(-1)
    qi = np.concatenate([-q[:, :3], q[:, 3:]], -1) / s[:, None]
    v, w = qi[:, :3], qi[:, 3]
    vv = v[:, :, None] * v[:, None, :]
    w2mv = w * w - (v * v).sum(-1)
    Vx = np.zeros((B, 3, 3))
    Vx[:, 0, 1] = -v[:, 2]
    Vx[:, 0, 2] = v[:, 1]
    Vx[:, 1, 0] = v[:, 2]
    Vx[:, 1, 2] = -v[:, 0]
    Vx[:, 2, 0] = -v[:, 1]
    Vx[:, 2, 1] = v[:, 0]
    R = (w2mv[:, None, None] * np.eye(3)
         + 2.0 * vv
         + 2.0 * w[:, None, None] * Vx)
    A = R / HALF_EXT[None, :, None]
    o = -(t + BOX_CENTER[None, :]) / HALF_EXT[None, :]
    bz = -t[:, 2] / HALF_EXT[0]       # sz-drain bias: -t_z / H_x
    return A.astype(np.float32), o.astype(np.float32), bz.astype(np.float32)


def _make_in_maps(trans, quat, pc):
    A, o, bz = _host_coefficients(trans, quat)
    coef_full = np.concatenate(
        [o, bz[:, None], np.zeros((B_FULL, 4), np.float32)], axis=1)  # [B,8]
    # planar fp16 [3, B, N]
    pcT = np.ascontiguousarray(
        np.asarray(pc, np.float32).transpose(2, 0, 1)).astype(np.float16)
    # diagonal weights [tile, i, c] per core
    idx = np.arange(128)
    in_maps = []
    for cidx in range(N_CORES):
        bs, be = cidx * B_CORE, (cidx + 1) * B_CORE
        Ac = A[bs:be].reshape(N_TILES, 128, 3, 3)
        wdc = np.zeros((N_TILES, 3, 3, 128, 128), np.float16)
        wdc[:, :, :, idx, idx] = np.transpose(
            Ac, (0, 2, 3, 1)).astype(np.float16)
        in_maps.append({
            "xyz": np.ascontiguousarray(pcT[:, bs:be, :]),
            "coef": np.ascontiguousarray(
                coef_full[bs:be].reshape(N_TILES, 128, 8)),
            "wd": np.ascontiguousarray(
                np.transpose(wdc.reshape(N_TILES, 9, 128, 128),
                             (0, 2, 1, 3)).reshape(N_TILES, 128, 9 * 128)),
        })
    return in_maps


def run_spmd(trans, quat, pc, **spmd_kwargs):
    """Shard, run on 8 cores, gather. Returns (output, BassKernelResults)."""
    in_maps = _make_in_maps(trans, quat, pc)
    res = run_bass_kernel_spmd(
        _get_nc(), in_maps, list(range(N_CORES)), **spmd_kwargs)
    outs = [res.results[i]["out"].T.reshape(B_CORE, 1)
            for i in range(N_CORES)]
    full = np.concatenate(outs, axis=0).astype(np.float32)
    return full, res


def kernel(trans, quat, pc):
    full, _ = run_spmd(trans, quat, pc)
    return full


# revision 51
# speedup vs baseline: 1.0010x; 1.0010x over previous
"""Trainium2 Bass kernel for CollisionDistanceEvaluator (segment_reduce).

Contract: kernel(**inputs) takes FULL inputs (trans [4096,3] f32,
quat [4096,4] f32, pc [4096,4096,3] f32) and returns the FULL output
[4096,1] f32, running the per-point work on 8 NeuronCores (pure data
parallel over the batch dim, 512 batches/core, 4x 128-batch tiles).

Math: the reference rotates pc by inv(quat) (unit quat -> rotation R),
translates by -trans, tests an axis-aligned box (center C, half ext H),
and takes the per-batch masked mean of point norms.  Host precomputes
per batch: A = R/H (box-normalized rotation rows), o = -(t+C)/H, and
bz = -t_z/Hx.  Device, per point p, with v_i = A_i.p + o_i:
    mask    = max(v_x^2, v_y^2, v_z^2) <= 1
    n2/Hx^2 = v_x^2 + ky*v_y^2 + kz*v_z^2 + kl*v_z + cc     (even pairs)
            = v_x^2 + ky*v_y^2 + ((Hz*v_z - t_z + ...)/Hx)^2 + ccxy (odd)
    out[b]  = -10000*Hx*sum(mask*sqrt(n2~))/max(cnt,1)  (+10000 if cnt=0)
(x/y-linear C-terms are dropped: validated scale_rel 2.8e-3 vs 2e-2 gate;
EPS_PAD keeps the f16-rounded n2~ nonnegative for Sqrt.)

Engine split per 2048-pt pair (2 PSUM chunks of 1024):
  PE  : 18 diag matmuls/chunk (z,x,y order, c-outer = 9 LDWEIGHTS)
  ACT : drains PSUM->SBUF f16 -- even pairs: Identity(vz), Square(qx,qy);
        odd pairs: Square(qz), Square(sz = exact z-term), Square(qx,qy)
        -- plus pair-wide Sqrt+accum, issued one stage late so the ACT
        queue never blocks on DVE
  DVE : qz=vz*vz (even), max-chain, is_le+count-accum, n2 assembly with
        4x-mode tensor_scalar ops + 2x tensor_tensor adds, mask*n2
The even/odd alternation balances ACT vs DVE load.  First/last pairs run
chunk-granular to shorten the pipeline ramp/tail; extra PE warmup keeps
the HAM clock hot until the first DMAs land; the result is stored as one
packed [128, N_TILES] f16 DMA (a [128,1]-per-tile store would scatter
128 descriptors each and tail the kernel).
"""

import numpy as np

import concourse.bass as bass
import concourse.bacc as bacc
import concourse.mybir as mybir
from concourse.tile import TileContext
from concourse.bass_utils import run_bass_kernel_spmd
from concourse import library_config


def _ensure_ntff_hook():
    """Register the axon NTFF profile hook if the image's antenv lacks it."""
    import sys
    import types
    try:
        from antenv.axon_hooks import get_axon_ntff_profile_hook  # noqa
        return
    except ImportError:
        pass
    try:
        import antenv
        from trn_agent_boot.trn_boot import _ntff_profile_via_ctypes
        mod = types.ModuleType("antenv.axon_hooks")
        mod._hook = _ntff_profile_via_ctypes("/opt/axon/libaxon_pjrt.so")

        def set_axon_ntff_profile_hook(h):
            mod._hook = h

        def get_axon_ntff_profile_hook():
            return mod._hook

        mod.set_axon_ntff_profile_hook = set_axon_ntff_profile_hook
        mod.get_axon_ntff_profile_hook = get_axon_ntff_profile_hook
        sys.modules["antenv.axon_hooks"] = mod
        antenv.axon_hooks = mod
    except Exception:
        pass


_ensure_ntff_hook()

N_CORES = 8
B_FULL, N_PTS = 4096, 4096
B_CORE = B_FULL // N_CORES          # 512
N_TILES = B_CORE // 128             # 4
CHUNK = 1024                         # points per PSUM/drain chunk
N_CHUNKS = N_PTS // CHUNK           # 4
PAIR = 2 * CHUNK                     # DVE/sqrt operate pair-wide
N_PAIRS = N_PTS // PAIR             # 2
MM = 512                             # matmul moving max

DIST_THRESHOLD = 0.001
DIST_COEFF = 10000.0
BOX_CENTER = np.array([-0.001782, 1.005e-05, 0.0431621], dtype=np.float64)
HALF_EXT = np.array([
    0.204416 / 2 + DIST_THRESHOLD,
    0.0632517 / 2 + DIST_THRESHOLD,
    0.1381738 / 2 + DIST_THRESHOLD,
], dtype=np.float64)

# n2 normalization constants (n2 / Hx^2)
_HX2 = HALF_EXT[0] * HALF_EXT[0]
K_Y = float((HALF_EXT[1] / HALF_EXT[0]) ** 2)
K_Z = float((HALF_EXT[2] / HALF_EXT[0]) ** 2)
KL = float(2.0 * HALF_EXT[2] * BOX_CENTER[2] / _HX2)  # v_z linear coeff
HZHX = float(HALF_EXT[2] / HALF_EXT[0])
EPS_PAD = 1.5e-3
CC = float((BOX_CENTER[0] ** 2 + BOX_CENTER[1] ** 2 + BOX_CENTER[2] ** 2)
           / _HX2 + EPS_PAD)
CC_XY = float((BOX_CENTER[0] ** 2 + BOX_CENTER[1] ** 2) / _HX2 + EPS_PAD)
OUT_SCALE = float(-DIST_COEFF * HALF_EXT[0])

_NC_CACHE = {}


def _build_bass():
    f16, f32 = mybir.dt.float16, mybir.dt.float32
    Alu = mybir.AluOpType
    Act = mybir.ActivationFunctionType

    nc = bacc.Bacc()
    xyz = nc.declare_dram_parameter(
        "xyz", [3, B_CORE, N_PTS], f16, isOutput=False)
    # per-batch scalars: cols 0-2 = o_i
    coef = nc.declare_dram_parameter(
        "coef", [N_TILES, 128, 8], f32, isOutput=False)
    # diagonal PE weights: [tile, i, c] -> diag(A[batch, i, c])
    wd = nc.declare_dram_parameter(
        "wd", [N_TILES, 128, 9 * 128], f16, isOutput=False)
    # out[p, t] = output of batch (t*128 + p); host transposes
    out = nc.declare_dram_parameter("out", [128, N_TILES], f16, isOutput=True)
    xyz_ap, coef_ap, wd_ap, out_ap = xyz[:], coef[:], wd[:], out[:]

    with TileContext(nc) as tc, \
            tc.tile_pool(name="data", bufs=2) as data, \
            tc.tile_pool(name="ck", bufs=3) as ck, \
            tc.tile_pool(name="wpool", bufs=2) as wpool, \
            tc.tile_pool(name="psum", bufs=1, space="PSUM") as psum, \
            tc.tile_pool(name="small", bufs=4) as small:
        # PE warm-up: junk matmuls during the initial DMA ramp keep the
        # HAM clock gate open so the first real matmuls run at 2.4 GHz
        wu = wpool.tile([128, 128], f16, tag="warm")
        nc.vector.memset(wu[:], 0.0)
        res16 = small.tile([128, N_TILES], f16, tag="res")
        pwu = psum.tile([128, CHUNK], f32, tag="uza")
        for k in range(32):
            nc.tensor.matmul(pwu[:, :128], wu[:], wu[:, :],
                             start=True, stop=True)

        for t in range(N_TILES):
            r = slice(t * 128, (t + 1) * 128)
            ct = small.tile([128, 8], f32, tag="coef")
            nc.sync.dma_start(out=ct[:], in_=coef_ap[t, :, :])
            xt = data.tile([128, N_PTS], f16, tag="x")
            yt = data.tile([128, N_PTS], f16, tag="y")
            zt = data.tile([128, N_PTS], f16, tag="z")
            # weights first (gate the first LDWEIGHTS), then the first
            # chunk's columns so compute starts early
            wt = wpool.tile([128, 9 * 128], f16, tag="wt")
            nc.sync.dma_start(out=wt[:], in_=wd_ap[t, :, :])
            q0 = slice(0, CHUNK)
            nc.sync.dma_start(out=zt[:, q0], in_=xyz_ap[2, r, q0])
            nc.sync.dma_start(out=xt[:, q0], in_=xyz_ap[0, r, q0])
            nc.sync.dma_start(out=yt[:, q0], in_=xyz_ap[1, r, q0])
            q1 = slice(CHUNK, 2 * CHUNK)
            nc.sync.dma_start(out=zt[:, q1], in_=xyz_ap[2, r, q1])
            nc.sync.dma_start(out=xt[:, q1], in_=xyz_ap[0, r, q1])
            nc.sync.dma_start(out=yt[:, q1], in_=xyz_ap[1, r, q1])
            h1 = slice(2 * CHUNK, N_PTS)
            nc.sync.dma_start(out=zt[:, h1], in_=xyz_ap[2, r, h1])
            nc.sync.dma_start(out=xt[:, h1], in_=xyz_ap[0, r, h1])
            nc.sync.dma_start(out=yt[:, h1], in_=xyz_ap[1, r, h1])
            planes = (xt, yt, zt)
            if t == 0:
                # keep the PE clock hot while the first DMAs land (a
                # >3.4us PE idle gap re-throttles the HAM clock)
                for k in range(56):
                    nc.tensor.matmul(pwu[:, :128], wu[:], wu[:, :],
                                     start=True, stop=True)

            def wsel(i, c, wt=wt):
                w = 3 * i + c
                return wt[:, w * 128:(w + 1) * 128]

            rcv = small.tile([128, N_CHUNKS], f32, tag="rcv")
            rsv = small.tile([128, N_CHUNKS], f32, tag="rsv")

            pend = []  # sqrt runs 1 stage late so ACT never stalls on DVE
            slot = [0]

            def flush_sqrt(keep):
                while len(pend) > keep:
                    pmn, ss = pend.pop(0)
                    w = pmn.shape[-1]
                    sq = ck.tile([128, PAIR], f16, tag="mx")
                    nc.scalar.activation(
                        sq[:, :w], pmn, Act.Sqrt,
                        accum_out=rsv[:, ss:ss + 1])

            def dve_stage(w, odd, qx, qy, qz, vz, sz, mn):
                # DVE mask chain + count accum on slice w of the pair
                if not odd:
                    nc.vector.tensor_tensor(qz[:, w], vz[:, w], vz[:, w],
                                            Alu.mult)
                mx = ck.tile([128, PAIR], f16, tag="mx")
                wd_ = w.stop - w.start
                mxv = mx[:, :wd_]
                nc.vector.tensor_tensor(mxv, qx[:, w], qy[:, w], Alu.max)
                nc.vector.tensor_tensor(mxv, mxv, qz[:, w], Alu.max)
                g = ck.tile([128, PAIR], f16, tag="g")
                gv = g[:, :wd_]
                ss = slot[0]
                slot[0] += 1
                nc.vector.tensor_scalar(
                    gv, mxv, 1.0, 0.0, Alu.is_le, Alu.add,
                    accum_out=rcv[:, ss:ss + 1])
                # DVE n2 assembly (4x ts + 2x TT):
                # even: n2~ = qx + ky*qy + kz*qz + kl*vz + cc
                # odd : n2~ = qx + (ky*qy + ccxy) + sz
                t1 = ck.tile([128, PAIR], f16, tag="t1")
                t1v = t1[:, :wd_]
                if odd:
                    nc.vector.tensor_scalar(
                        t1v, qy[:, w], K_Y, CC_XY, Alu.mult, Alu.add)
                    nc.vector.tensor_tensor(t1v, qx[:, w], t1v, Alu.add)
                    nc.vector.tensor_tensor(t1v, t1v, sz[:, w], Alu.add)
                else:
                    nc.vector.tensor_scalar(
                        t1v, qy[:, w], K_Y, None, Alu.mult)
                    t2 = ck.tile([128, PAIR], f16, tag="t2")
                    t2v = t2[:, :wd_]
                    s3 = ck.tile([128, PAIR], f16, tag="s3")
                    s3v = s3[:, :wd_]
                    nc.vector.tensor_scalar(
                        t2v, qz[:, w], K_Z, None, Alu.mult)
                    nc.vector.tensor_scalar(
                        s3v, vz[:, w], KL, CC, Alu.mult, Alu.add)
                    nc.vector.tensor_tensor(t1v, qx[:, w], t1v, Alu.add)
                    nc.vector.tensor_tensor(t2v, t2v, s3v, Alu.add)
                    nc.vector.tensor_tensor(t1v, t1v, t2v, Alu.add)
                # masked n2 (g is the 0/1 mask)
                mn = ck.tile([128, PAIR], f16, tag="mn")
                mnv = mn[:, :wd_]
                nc.vector.tensor_tensor(mnv, gv, t1v, Alu.mult)
                pend.append((mnv, ss))

            for p in range(N_PAIRS):
                odd = (t * N_PAIRS + p) % 2 == 1
                # first pair of the first tile and last pair of the last
                # tile run the DVE stage per chunk: shorter ramp/tail
                split = (t == 0 and p == 0) or \
                        (t == N_TILES - 1 and p == N_PAIRS - 1)
                # pair-wide drained tiles, filled chunk-halves at a time
                qx = ck.tile([128, PAIR], f16, tag="qx")
                qy = ck.tile([128, PAIR], f16, tag="qy")
                qz = ck.tile([128, PAIR], f16, tag="qz")
                sz = vz = None
                if odd:
                    sz = ck.tile([128, PAIR], f16, tag="t2")
                else:
                    vz = ck.tile([128, PAIR], f16, tag="vz")
                for h2 in range(2):
                    j = 2 * p + h2
                    d = slice(h2 * CHUNK, (h2 + 1) * CHUNK)
                    uz = psum.tile([128, CHUNK], f32,
                                   tag="uza" if j % 2 == 0 else "uzb")
                    ux = psum.tile([128, CHUNK], f32, tag="ux")
                    uy = psum.tile([128, CHUNK], f32, tag="uy")
                    # PE: z first (its drains feed the DVE chain); c outer
                    # so each diag weight loads once per chunk (9 LDW)
                    for ut, i in ((uz, 2), (ux, 0), (uy, 1)):
                        for c in range(3):
                            for h in range(CHUNK // MM):
                                hs = slice(h * MM, (h + 1) * MM)
                                ps = slice(j * CHUNK + h * MM,
                                           j * CHUNK + (h + 1) * MM)
                                nc.tensor.matmul(
                                    ut[:, hs], wsel(i, c),
                                    planes[c][:, ps],
                                    start=(c == 0), stop=(c == 2))
                    # ACT drains (PSUM -> SBUF f16).  Even pairs: z as
                    # Identity (DVE squares it); odd pairs: z drained twice
                    # on ACT (qz mask + sz = exact z term of n2~) so DVE
                    # and ACT loads balance.
                    if odd:
                        nc.scalar.activation(
                            qz[:, d], uz[:], Act.Square, bias=ct[:, 2:3])
                        nc.scalar.activation(
                            sz[:, d], uz[:], Act.Square, bias=ct[:, 3:4],
                            scale=HZHX)
                    else:
                        nc.scalar.activation(
                            vz[:, d], uz[:], Act.Identity, bias=ct[:, 2:3])
                    nc.scalar.activation(
                        qx[:, d], ux[:], Act.Square, bias=ct[:, 0:1])
                    nc.scalar.activation(
                        qy[:, d], uy[:], Act.Square, bias=ct[:, 1:2])
                    if split:
                        dve_stage(d, odd, qx, qy, qz, vz, sz, None)
                    if h2 == 0:
                        flush_sqrt(1)
                if not split:
                    dve_stage(slice(0, PAIR), odd, qx, qy, qz, vz,
                              sz, None)
            flush_sqrt(0)

            # tail: out = (cnt==0)*10000 + OUT_SCALE*rs/max(cnt,1)
            rc = small.tile([128, 1], f32, tag="rc")
            rs = small.tile([128, 1], f32, tag="rs")
            ns = slot[0]
            nc.vector.tensor_reduce(
                rc[:], rcv[:, 0:ns], mybir.AxisListType.X, Alu.add)
            nc.vector.tensor_reduce(
                rs[:], rsv[:, 0:ns], mybir.AxisListType.X, Alu.add)
            rc1 = small.tile([128, 1], f32, tag="rc1")
            nc.vector.tensor_scalar(rc1[:], rc[:], 1.0, None, Alu.max)
            inv = small.tile([128, 1], f32, tag="inv")
            nc.vector.reciprocal(inv[:], rc1[:])
            val = small.tile([128, 1], f32, tag="val")
            nc.vector.scalar_tensor_tensor(
                val[:], rs[:], OUT_SCALE, inv[:], Alu.mult, Alu.mult)
            zer = small.tile([128, 1], f32, tag="zer")
            nc.vector.tensor_scalar(zer[:], rc[:], 0.0, None, Alu.is_le)
            nc.vector.scalar_tensor_tensor(
                res16[:, t:t + 1], zer[:], DIST_COEFF, val[:],
                Alu.mult, Alu.add)

        # one packed result store (vs four 128-descriptor scatter DMAs)
        nc.sync.dma_start(out=out_ap[:, :], in_=res16[:])
    nc.compile()
    return nc


def _get_nc():
    if "nc" not in _NC_CACHE:
        _NC_CACHE["nc"] = _build_bass()
    return _NC_CACHE["nc"]


def _host_coefficients(trans, quat):
    """Per-batch A = R/H [B,3,3] and o = -(t+C)/H [B,3] (computed in f64)."""
    q = np.asarray(quat, np.float64)
    t = np.asarray(trans, np.float64)
    B = q.shape[0]
    s = (q * q).sum(-1)
    qi = np.concatenate([-q[:, :3], q[:, 3:]], -1) / s[:, None]
    v, w = qi[:, :3], qi[:, 3]
    vv = v[:, :, None] * v[:, None, :]
    w2mv = w * w - (v * v).sum(-1)
    Vx = np.zeros((B, 3, 3))
    Vx[:, 0, 1] = -v[:, 2]
    Vx[:, 0, 2] = v[:, 1]
    Vx[:, 1, 0] = v[:, 2]
    Vx[:, 1, 2] = -v[:, 0]
    Vx[:, 2, 0] = -v[:, 1]
    Vx[:, 2, 1] = v[:, 0]
    R = (w2mv[:, None, None] * np.eye(3)
         + 2.0 * vv
         + 2.0 * w[:, None, None] * Vx)
    A = R / HALF_EXT[None, :, None]
    o = -(t + BOX_CENTER[None, :]) / HALF_EXT[None, :]
    bz = -t[:, 2] / HALF_EXT[0]       # sz-drain bias: -t_z / H_x
    return A.astype(np.float32), o.astype(np.float32), bz.astype(np.float32)


def _make_in_maps(trans, quat, pc):
    A, o, bz = _host_coefficients(trans, quat)
    coef_full = np.concatenate(
        [o, bz[:, None], np.zeros((B_FULL, 4), np.float32)], axis=1)  # [B,8]
    # planar fp16 [3, B, N]
    pcT = np.ascontiguousarray(
        np.asarray(pc, np.float32).transpose(2, 0, 1)).astype(np.float16)
    # diagonal weights [tile, i, c] per core
    idx = np.arange(128)
    in_maps = []
    for cidx in range(N_CORES):
        bs, be = cidx * B_CORE, (cidx + 1) * B_CORE
        Ac = A[bs:be].reshape(N_TILES, 128, 3, 3)
        wdc = np.zeros((N_TILES, 3, 3, 128, 128), np.float16)
        wdc[:, :, :, idx, idx] = np.transpose(
            Ac, (0, 2, 3, 1)).astype(np.float16)
        in_maps.append({
            "xyz": np.ascontiguousarray(pcT[:, bs:be, :]),
            "coef": np.ascontiguousarray(
                coef_full[bs:be].reshape(N_TILES, 128, 8)),
            "wd": np.ascontiguousarray(
                np.transpose(wdc.reshape(N_TILES, 9, 128, 128),
                             (0, 2, 1, 3)).reshape(N_TILES, 128, 9 * 128)),
        })
    return in_maps


def run_spmd(trans, quat, pc, **spmd_kwargs):
    """Shard, run on 8 cores, gather. Returns (output, BassKernelResults)."""
    in_maps = _make_in_maps(trans, quat, pc)
    res = run_bass_kernel_spmd(
        _get_nc(), in_maps, list(range(N_CORES)), **spmd_kwargs)
    outs = [res.results[i]["out"].T.reshape(B_CORE, 1)
            for i in range(N_CORES)]
    full = np.concatenate(outs, axis=0).astype(np.float32)
    return full, res


def kernel(trans, quat, pc):
    full, _ = run_spmd(trans, quat, pc)
    return full
